# revision 12
# baseline (speedup 1.0000x reference)
# GRU encoder kernel for Trainium2 (Bass/Tile), data-parallel over batch on 8 cores.
#
# Model (per reference):
#   x  = embedding[enc_inputs]                      [B, T, 100]
#   h0 = [labels @ W1 + b1, zeros]                  [B, 700]
#   xp = x @ Wx + b_in                              [T, B, 2100]
#   scan t: rec = h @ Wh + b_rec                    [B, 2100]
#           z = sig(xp_z + rec_z); r = sig(xp_r + rec_r)
#           hh = tanh(xp_h + r * rec_h); h = z*h + (1-z)*hh
#   out = h[:, 200:700]
#
# Sharding: batch 256 -> 32 rows per core, weights replicated, no collectives.
#
# Per-core layout: hidden padded 700->704 (3 blocks of 704 = 2112 cols).
# The recurrent matmul keeps batch (32) on PSUM partitions and streams
# Wh through the PE. The contraction is augmented so PSUM directly holds
# the gate pre-activations:
#   k=0..5 : lhsT = h^T chunks (k=5 also carries a ones-row -> + b_rec)
#   k=6    : lhsT = I32, rhs = xp_t  (z/r blocks only -> + xp)
# h^T is rebuilt each step with 6 PE transpose ops (M=32 -> cheap).

import os
import sys
from contextlib import ExitStack

import numpy as np

if "/opt/trn_rl_repo" not in sys.path:
    sys.path.insert(0, "/opt/trn_rl_repo")

import concourse.bass as bass
import concourse.mybir as mybir
import concourse.tile as tile
from concourse import bacc
from concourse.bass_utils import run_bass_kernel_spmd
from concourse.masks import make_identity

F32 = mybir.dt.float32
I32DT = mybir.dt.int32
AF = mybir.ActivationFunctionType

P = 128
VOCAB, EMB = 30000, 100
DIM_Y, DIM_Z = 200, 500
H = 700
HP = 704                    # padded hidden block
W3 = 3 * HP                 # 2112
B, T_FULL = 256, 256
NCORES = 8
BL = B // NCORES            # 32 rows per core
KT = 6                      # hidden K tiles: 5 x 128 + (64 + bias row)
CHUNKS = ((0, 512), (512, HP - 512))   # PSUM-bank-aligned column chunks of a block


def _lhsT_k(hT, k):
    # weight (stationary) operand for hidden K-tile k: h^T chunk.
    if k < 5:
        return hT[0:P, k * 32:(k + 1) * 32]
    # rows 640:704 of h^T plus the ones-row (row 64) that injects b_rec
    return hT[0:65, 5 * 32:6 * 32]


def emit_gru(ctx, tc, io, T):
    nc = tc.nc
    enc, emb, whd, wxd = io["enc"], io["emb"], io["wh"], io["wx"]
    labd, w1d, out_d = io["lab"], io["w1b"], io["out"]

    tcs = min(P, T)               # timesteps per gather/matmul tile
    ntc = (T + tcs - 1) // tcs    # t-chunks

    # scratch DRAM for the precomputed input projections, scan-friendly layout
    xpzr_d = nc.dram_tensor("xpzr", [T, BL, 2 * HP], F32, kind="Internal").ap()
    xph_d = nc.dram_tensor("xph", [T, BL, HP], F32, kind="Internal").ap()

    const = ctx.enter_context(tc.tile_pool(name="const", bufs=1))

    ident = const.tile([P, P], F32, name="ident")
    make_identity(nc, ident[:])

    # static weights in SBUF
    wh_sb = const.tile([P, KT * W3], F32, name="wh_sb")
    for k in range(KT):
        nc.sync.dma_start(wh_sb[:, k * W3:(k + 1) * W3], whd[k])
    wx_sb = const.tile([EMB + 1, W3], F32, name="wx_sb")
    nc.sync.dma_start(wx_sb[:], wxd[:])

    # token ids, laid out so gather offsets are SBUF column slices
    enc_sb = const.tile([tcs, ntc * BL], I32DT, name="enc_sb")
    for c in range(ntc):
        nc.sync.dma_start(
            enc_sb[:, c * BL:(c + 1) * BL], enc[c * tcs:(c + 1) * tcs, :]
        )

    lab_sb = const.tile([2, BL], F32, name="lab_sb")
    nc.sync.dma_start(lab_sb[:], labd[:])
    w1_sb = const.tile([2, DIM_Y], F32, name="w1_sb")
    nc.sync.dma_start(w1_sb[:], w1d[:])

    # hidden state (ping-pong), batch-major and transposed
    h_t = [const.tile([BL, HP], F32, name=f"h{i}") for i in range(2)]
    hT_t = [const.tile([P, KT * 32], F32, name=f"hT{i}") for i in range(2)]
    for i in range(2):
        nc.gpsimd.memset(h_t[i][:], 0.0)
        nc.gpsimd.memset(hT_t[i][:], 0.0)
        # ones row of hT K-tile 5 -> adds b_rec to every gate block
        nc.vector.memset(hT_t[i][64:65, 5 * 32:6 * 32], 1.0)

    # x^T tiles for the input projection (ping-pong); row 100 = ones -> + b_in
    # (engines need 32-aligned partition bases, so row 100 is written via an
    # affine_select on the [96:128] partition group: 1.0 where x - 4 == 0)
    xt_sb = [const.tile([P, tcs], F32, name=f"xt{i}") for i in range(2)]
    for i in range(2):
        nc.gpsimd.memset(xt_sb[i][:], 0.0)
        nc.gpsimd.affine_select(
            out=xt_sb[i][96:P, :],
            in_=xt_sb[i][96:P, :],
            compare_op=mybir.AluOpType.not_equal,
            fill=1.0,
            base=-4,
            channel_multiplier=1,
            pattern=[[0, tcs]],
        )

    def emit_transposes(h_src, hT_dst, ks, pool, tag="tr"):
        for k in ks:
            ck = 128 if k < 5 else HP - 5 * 128
            trp = pool.tile([P, 32], F32, tag=tag, name=f"tr{k}")
            nc.tensor.transpose(
                trp[0:ck, 0:32], h_src[:, k * 128:k * 128 + ck], ident[0:BL, 0:BL]
            )
            cp = nc.scalar.copy if k % 2 else nc.vector.tensor_copy
            cp(hT_dst[0:ck, k * 32:(k + 1) * 32], trp[0:ck, 0:32])

    # ---------------- phase A+B: h0 and input projections ----------------
    with tc.tile_pool(name="ps_b", bufs=1, space="PSUM") as ps_big, \
         tc.tile_pool(name="ps_s", bufs=2, space="PSUM") as ps_small, \
         tc.tile_pool(name="sb_b", bufs=2) as sb_b:

        # h0 = [labels x W1 + b1, 0]
        h0_ps = ps_small.tile([BL, DIM_Y], F32, tag="small", name="h0ps")
        nc.tensor.matmul(h0_ps[:], lab_sb[:], w1_sb[:], start=True, stop=True)
        nc.vector.tensor_copy(h_t[0][:, 0:DIM_Y], h0_ps[:])
        emit_transposes(h_t[0], hT_t[0], range(KT), ps_small, tag="small")

        # xp = [x ; 1] @ [Wx ; b_in], staged to DRAM in scan order
        for c in range(ntc):
            for b in range(BL):
                pp = c * BL + b
                xg = sb_b.tile([tcs, EMB], F32, tag="xg", name=f"xg{pp}")
                nc.gpsimd.indirect_dma_start(
                    out=xg[:],
                    out_offset=None,
                    in_=emb[:],
                    in_offset=bass.IndirectOffsetOnAxis(
                        ap=enc_sb[:, c * BL + b:c * BL + b + 1], axis=0
                    ),
                )
                xt_ps = ps_small.tile([EMB, tcs], F32, tag="small", name=f"xtp{pp}")
                nc.tensor.transpose(xt_ps[:], xg[:], ident[0:tcs, 0:tcs])
                xt = xt_sb[pp % 2]
                nc.vector.tensor_copy(xt[0:EMB, :], xt_ps[:])

                xp_ps = ps_big.tile([tcs, W3], F32, tag="xp", name=f"xpp{pp}")
                for o in range(0, W3, 512):
                    n = min(512, W3 - o)
                    nc.tensor.matmul(
                        xp_ps[:, o:o + n], xt[0:EMB + 1, 0:tcs], wx_sb[:, o:o + n],
                        start=True, stop=True,
                    )
                xp_sb = sb_b.tile([tcs, W3], F32, tag="xps", name=f"xps{pp}")
                nc.vector.tensor_copy(xp_sb[:, 0:1024], xp_ps[:, 0:1024])
                nc.scalar.copy(xp_sb[:, 1024:W3], xp_ps[:, 1024:W3])
                nc.sync.dma_start(
                    xpzr_d[c * tcs:(c + 1) * tcs, b, :], xp_sb[:, 0:2 * HP]
                )
                nc.sync.dma_start(
                    xph_d[c * tcs:(c + 1) * tcs, b, :], xp_sb[:, 2 * HP:W3]
                )

    # ---------------- phase C: the scan ----------------
    with tc.tile_pool(name="ps_g", bufs=1, space="PSUM") as ps_gates, \
         tc.tile_pool(name="ps_tr", bufs=2, space="PSUM") as ps_tr, \
         tc.tile_pool(name="xp_pool", bufs=3) as xp_pool, \
         tc.tile_pool(name="sb_g", bufs=2) as sb_g:

        def mm_block(ps, hT, blk, xzr, k_outer):
            # accumulate one gate block (z:0, r:1, g:2) into PSUM
            nk = 6 if blk == 2 else 7
            order = (
                [(k, c) for k in range(nk) for c in range(2)]
                if k_outer else
                [(k, c) for c in range(2) for k in range(nk)]
            )
            for k, c in order:
                o, n = CHUNKS[c]
                if k < KT:
                    kp = P if k < 5 else 65
                    nc.tensor.matmul(
                        ps[:, o:o + n],
                        _lhsT_k(hT, k),
                        wh_sb[0:kp, k * W3 + blk * HP + o: k * W3 + blk * HP + o + n],
                        start=(k == 0), stop=(k == nk - 1),
                    )
                else:
                    nc.tensor.matmul(
                        ps[:, o:o + n],
                        ident[0:BL, 0:BL],
                        xzr[:, blk * HP + o: blk * HP + o + n],
                        start=False, stop=True,
                    )

        for t in range(T):
            cur = t % 2
            h, hT = h_t[cur], hT_t[cur]
            hn, hTn = h_t[1 - cur], hT_t[1 - cur]

            xzr = xp_pool.tile([BL, 2 * HP], F32, tag="xzr", name=f"xzr{t}")
            nc.sync.dma_start(xzr[:], xpzr_d[t])
            xh = xp_pool.tile([BL, HP], F32, tag="xh", name=f"xh{t}")
            nc.sync.dma_start(xh[:], xph_d[t])

            r_ps = ps_gates.tile([BL, HP], F32, tag="r_ps", name=f"rps{t}")
            g_ps = ps_gates.tile([BL, HP], F32, tag="g_ps", name=f"gps{t}")
            z_ps = ps_gates.tile([BL, HP], F32, tag="z_ps", name=f"zps{t}")

            # r first (needed by the tanh path), k-outer so K-tile 0 can start
            # as soon as the first transposed chunk of h is ready
            mm_block(r_ps, hT, 1, xzr, k_outer=True)
            r_sb = sb_g.tile([BL, HP], F32, tag="r_sb", name=f"rsb{t}")
            nc.scalar.activation(r_sb[:], r_ps[:], AF.Sigmoid)

            mm_block(g_ps, hT, 2, xzr, k_outer=False)
            q = sb_g.tile([BL, HP], F32, tag="q", name=f"q{t}")
            t2 = sb_g.tile([BL, HP], F32, tag="t2", name=f"t2{t}")
            hh = sb_g.tile([BL, HP], F32, tag="hh", name=f"hh{t}")
            for o, n in CHUNKS:
                nc.vector.tensor_mul(q[:, o:o + n], r_sb[:, o:o + n], g_ps[:, o:o + n])
                nc.vector.tensor_add(t2[:, o:o + n], q[:, o:o + n], xh[:, o:o + n])
                nc.scalar.activation(hh[:, o:o + n], t2[:, o:o + n], AF.Tanh)

            mm_block(z_ps, hT, 0, xzr, k_outer=False)
            z_sb = sb_g.tile([BL, HP], F32, tag="z_sb", name=f"zsb{t}")
            nc.scalar.activation(z_sb[:], z_ps[:], AF.Sigmoid)

            # h' = hh + z * (h - hh), chunked; transpose chunks as they land
            d = sb_g.tile([BL, HP], F32, tag="d", name=f"d{t}")
            e = sb_g.tile([BL, HP], F32, tag="e", name=f"e{t}")
            for ci, (o, n) in enumerate(CHUNKS):
                nc.vector.tensor_sub(d[:, o:o + n], h[:, o:o + n], hh[:, o:o + n])
                nc.vector.tensor_mul(e[:, o:o + n], d[:, o:o + n], z_sb[:, o:o + n])
                nc.vector.tensor_add(hn[:, o:o + n], e[:, o:o + n], hh[:, o:o + n])
                emit_transposes(
                    hn, hTn, range(0, 4) if ci == 0 else range(4, KT), ps_tr
                )

        nc.sync.dma_start(out_d[:], h_t[T % 2][:, DIM_Y:H])


def build_core_program(T=T_FULL):
    nc = bacc.Bacc("TRN2", target_bir_lowering=False, debug=False)
    io = {
        "enc": nc.dram_tensor("enc", [T, BL], I32DT, kind="ExternalInput").ap(),
        "emb": nc.dram_tensor("emb", [VOCAB, EMB], F32, kind="ExternalInput").ap(),
        "wh": nc.dram_tensor("wh", [KT, P, W3], F32, kind="ExternalInput").ap(),
        "wx": nc.dram_tensor("wx", [EMB + 1, W3], F32, kind="ExternalInput").ap(),
        "lab": nc.dram_tensor("lab", [2, BL], F32, kind="ExternalInput").ap(),
        "w1b": nc.dram_tensor("w1b", [2, DIM_Y], F32, kind="ExternalInput").ap(),
        "out": nc.dram_tensor("out", [BL, DIM_Z], F32, kind="ExternalOutput").ap(),
    }
    with tile.TileContext(nc) as tc:
        with ExitStack() as ctx:
            emit_gru(ctx, tc, io, T)
    nc.compile()
    return nc


def pack_weights(Wx, Wh, bias, b1_unused=None):
    """Host-side layout staging (padding/stacking only, no compute)."""
    f = np.float32
    whp = np.zeros((HP + 1, W3), dtype=f)       # padded [hidden+biasrow, 3 blocks]
    brow = np.zeros((W3,), dtype=f)
    for blk in range(3):
        whp[:H, blk * HP:blk * HP + H] = Wh[:, blk * H:(blk + 1) * H]
        brow[blk * HP:blk * HP + H] = bias[1][blk * H:(blk + 1) * H]
    wh_aug = np.zeros((KT, P, W3), dtype=f)
    for k in range(5):
        wh_aug[k] = whp[k * P:(k + 1) * P]
    wh_aug[5, 0:64] = whp[640:HP]
    wh_aug[5, 64] = brow

    wx_aug = np.zeros((EMB + 1, W3), dtype=f)
    for blk in range(3):
        wx_aug[:EMB, blk * HP:blk * HP + H] = Wx[:, blk * H:(blk + 1) * H]
        wx_aug[EMB, blk * HP:blk * HP + H] = bias[0][blk * H:(blk + 1) * H]
    return wh_aug, wx_aug


_NC_CACHE = {}


def kernel(enc_inputs, labels, embedding, W1, b1, Wx, Wh, bias, _trace=False):
    T = enc_inputs.shape[1]
    if T not in _NC_CACHE:
        _NC_CACHE[T] = build_core_program(T)
    nc = _NC_CACHE[T]

    wh_aug, wx_aug = pack_weights(
        np.asarray(Wx, np.float32), np.asarray(Wh, np.float32),
        np.asarray(bias, np.float32),
    )
    emb = np.ascontiguousarray(np.asarray(embedding, np.float32))
    w1b = np.ascontiguousarray(
        np.stack([np.asarray(W1, np.float32)[0], np.asarray(b1, np.float32)])
    )

    in_maps = []
    for c in range(NCORES):
        sl = slice(c * BL, (c + 1) * BL)
        enc_t = np.ascontiguousarray(np.asarray(enc_inputs, np.int32)[sl].T)
        lab2 = np.ascontiguousarray(
            np.stack([np.asarray(labels, np.float32)[sl], np.ones(BL, np.float32)])
        )
        in_maps.append({
            "enc": enc_t, "emb": emb, "wh": wh_aug, "wx": wx_aug,
            "lab": lab2, "w1b": w1b,
        })

    res = run_bass_kernel_spmd(
        nc, in_maps, core_ids=list(range(NCORES)), trace=_trace,
    )
    out = np.concatenate([r["out"] for r in res.results], axis=0)
    if _trace:
        return out, res
    return out
